# revision 15
# baseline (speedup 1.0000x reference)
"""v6 = lane-split engine balancing + halo-baked DRAM band layout for d:
  d_halo[branch][rowhalf]: [128 slots=(band,ch), BR+16 rows, W+18 cols] bf16
  with the halo rows duplicated at write time, so each phase-B tile load is
  a single 3-dim DMA (vs 16 per-band DMAs that serialized the lanes).
v4/v5 core:
  - scalar_tensor_tensor chains (u/a/dot/y16) rewritten as ACT per-partition
    scaled copies + tensor_tensor adds (STT has no 2x DVE uop; ACT is idle),
  - tensor_tensor ops greedily split between DVE and GPSIMD by a cost-model
    load balancer (DVE bf16 TT ~1187ns vs GPSIMD ~2939ns per [128,2048] op).

v2 base: PE-based depthwise conv (8-shift stack, all 4 branches in one M=64
matmul accumulation over 7 row-offsets) + bf16 elementwise phase with
alignment-preserving dual d-tiles (dt / dt2 shifted by one column).

Layouts:
  x_dram [16, 272, 272+2] bf16, image at rows/cols [8, 264), zero halo.
  stack  [128 = 8 col-shifts x 16ch, 38 rows, 256] bf16 per 32-row strip.
  d_dram (per branch) [16, 272, 274] bf16, image at rows [8,264) cols [8,264),
         col pads zeroed; computed d = dw conv + bias (ACT fused).
  phase B: partitions = 8 row-bands x 16 ch; dt [128, BR+16, CW+16] plus
         dt2 (columns shifted +1) so every tensor_tensor operand keeps a
         4-byte-aligned start -> DVE bf16 2x mode stays engaged.
"""

import sys

sys.path.insert(0, "/opt/trn_rl_repo")

import numpy as np

import concourse.bass as bass  # noqa: E402
import concourse.mybir as mybir  # noqa: E402
from concourse import bacc  # noqa: E402
from concourse.tile import TileContext  # noqa: E402

F32 = mybir.dt.float32
BF16 = mybir.dt.bfloat16
AL = mybir.AluOpType
AF = mybir.ActivationFunctionType

SHIFTS = [1, 3, 5, 7]
OFFSETS = [(-1, -1), (-1, 0), (-1, 1), (0, 1), (1, 1), (1, 0), (1, -1), (0, -1)]
C = 16
PAD = 8  # halo width in x_dram / d_dram

BATCHER8 = [
    (0, 1), (2, 3), (4, 5), (6, 7),
    (0, 2), (1, 3), (4, 6), (5, 7),
    (1, 2), (5, 6),
    (0, 4), (1, 5), (2, 6), (3, 7),
    (2, 4), (3, 5),
    (1, 2), (3, 4), (5, 6),
]
SORT4 = [(0, 1), (2, 3), (0, 2), (1, 3), (1, 2)]


def pack_weights(w):
    c_of_p = np.arange(128) % C
    out = {}

    in_w = np.asarray(w["in_conv_w"], np.float32)
    lhsT = np.zeros((128, 32), np.float32)
    lhsT[0:64, 0:16] = in_w.T
    lhsT[64:128, 16:32] = in_w.T
    out["w_in"] = lhsT
    b2 = np.zeros((32, 1), np.float32)
    b2[0:16, 0] = np.asarray(w["in_conv_b"], np.float32)
    b2[16:32, 0] = np.asarray(w["in_conv_b"], np.float32)
    out["b_in"] = b2

    # stack-conv lhsT: [128=(g,c), 7 deltas x 64=(si,c')] bf16
    # g encodes column shift j = g-3; delta is the row offset (-3..3).
    dwL = np.zeros((128, 7 * 64), np.float32)
    for g in range(8):
        j = g - 3
        for c in range(C):
            p_row = g * C + c
            for dlt in range(-3, 4):
                for si, s in enumerate(SHIFTS):
                    p = s // 2
                    if abs(dlt) <= p and abs(j) <= p:
                        ww = np.asarray(w[f"dw_w{s}"], np.float32).reshape(C, s, s)
                        dwL[p_row, (dlt + 3) * 64 + si * C + c] = ww[c, dlt + p, j + p]
    out["dwL"] = dwL.astype(np.float32)  # cast at SBUF load

    dwB64 = np.zeros((64, 1), np.float32)
    for si, s in enumerate(SHIFTS):
        dwB64[si * C:(si + 1) * C, 0] = np.asarray(w[f"dw_b{s}"], np.float32)
    out["dwB64"] = dwB64

    l1w = np.zeros((128, 16), np.float32)
    l1b = np.zeros((128, 4), np.float32)
    l2w = np.zeros((128, 32), np.float32)
    l2b = np.zeros((128, 4), np.float32)
    w1 = np.asarray(w["l1_w"], np.float32)
    b1 = np.asarray(w["l1_b"], np.float32)
    w2 = np.asarray(w["l2_w"], np.float32)
    bb2 = np.asarray(w["l2_b"], np.float32)
    for si in range(4):
        for f in range(4):
            l1w[:, 4 * si + f] = w1[si, c_of_p, f]
        l1w[:, 4 * si + 3] *= 2.0
        l1b[:, si] = b1[si, c_of_p]
        for r in range(8):
            l2w[:, 8 * si + r] = w2[si, c_of_p, r]
        l2b[:, si] = bb2[si, c_of_p]
    out["l1w"] = l1w
    out["l1b"] = l1b
    out["l2w"] = l2w
    out["l2b"] = l2b

    bw = np.asarray(w["base_w"], np.float32)
    basew = np.zeros((128, 4), np.float32)
    for f in range(4):
        basew[:, f] = bw[c_of_p, f]
    out["basew"] = basew

    bn = np.zeros((128, 2), np.float32)
    bn[:, 0] = np.asarray(w["bn_scale"], np.float32)[c_of_p]
    bn[:, 1] = np.asarray(w["bn_bias"], np.float32)[c_of_p]
    out["bn"] = bn

    fw = np.asarray(w["final_w"], np.float32).reshape(C)
    fin = np.zeros((128, 8), np.float32)
    for p in range(128):
        fin[p, p // C] = fw[c_of_p[p]]
    out["fin"] = fin
    out["finb"] = np.full((8, 1), np.asarray(w["final_b"]).reshape(-1)[0], np.float32)
    return out


WSHAPES = {
    "w_in": (128, 32), "b_in": (32, 1), "dwL": (128, 448), "dwB64": (64, 1),
    "l1w": (128, 16), "l1b": (128, 4), "l2w": (128, 32), "l2b": (128, 4),
    "basew": (128, 4), "bn": (128, 2), "fin": (128, 8), "finb": (8, 1),
}
# which SBUF weight tiles are bf16 (matmul operands against bf16 rhs)
WBF16 = {"dwL", "fin", "w_in"}




def emit(nc, cen_ap, waps, out_ap, H, W, RH, CC, SR=32):
    BR = H // (8 * RH)
    CW = W // CC
    Wx = W + 2 * PAD            # x_dram width (272)
    Wd = W + 2 * PAD + 2        # d_dram width (274, even row stride + dt2 room)
    Hp = H + 2 * PAD
    Ph = H * W // 2
    rpc = 512 // W
    nchunks = Ph // 512
    SR = min(SR, H // 2)
    nstrips = H // SR
    assert SR % rpc == 0 and (SR * W) % 512 == 0

    with TileContext(nc) as tc:
        with tc.tile_pool(name="wp", bufs=1) as wp, \
             tc.tile_pool(name="ps", bufs=2, space="PSUM") as psp, \
             tc.tile_pool(name="dr", bufs=1, space="DRAM") as drp:

            wsb = {}
            for nm, shp in WSHAPES.items():
                dt_ = BF16 if nm in WBF16 else F32
                t = wp.tile(list(shp), dt_, name=f"wsb_{nm}", tag=f"w_{nm}")
                nc.gpsimd.dma_start(t, waps[nm])  # SWDGE: casts f32 -> bf16
                wsb[nm] = t

            # per-strip x tensors with 8-row halo baked in (rows r = image
            # row 32*sp - 8 + r), so depthwise strips start before in_conv ends
            x_halo = [drp.tile([16, SR + 16, Wx], BF16, name=f"xh{sp}",
                               tag=f"xh{sp}") for sp in range(H // SR)]
            # halo-baked band layout: slot p = (band%8)*16 + ch, rows BR+16
            SLOTR = BR + 16
            NB = 8 * RH          # global bands
            d_halo = [[drp.tile([128, SLOTR, Wd], BF16, name=f"dh{si}_{g}",
                                tag=f"dh{si}_{g}") for g in range(RH)]
                      for si in range(4)]

            # ---------------- phase A ----------------
            with tc.tile_pool(name="pa", bufs=2) as pa:
                # zero halos of the x strip tensors
                zrow = pa.tile([16, PAD, Wx], BF16, name="zrow", tag="zrow", bufs=1)
                zcol = pa.tile([16, SR + 16, PAD], BF16, name="zcol", tag="zcol",
                               bufs=1)
                nc.vector.memset(zrow, 0.0)
                nc.vector.memset(zcol, 0.0)
                nsp = H // SR
                for sp in range(nsp):
                    nc.sync.dma_start(x_halo[sp][:, :, 0:PAD], zcol)
                    nc.sync.dma_start(x_halo[sp][:, :, W + PAD:Wx], zcol)
                nc.sync.dma_start(x_halo[0][:, 0:PAD, :], zrow)
                nc.sync.dma_start(x_halo[nsp - 1][:, SR + PAD:SR + 16, :], zrow)
                # zero pads of the d_halo tensors: side cols + edge rows
                zpad = pa.tile([128, SLOTR, PAD + 10], BF16, name="zpad",
                               tag="zpad", bufs=1)
                zedge = pa.tile([16, PAD, Wd], BF16, name="zedge", tag="zedge",
                                bufs=1)
                nc.vector.memset(zpad, 0.0)
                nc.vector.memset(zedge, 0.0)
                for si in range(4):
                    for g in range(RH):
                        dd = d_halo[si][g]
                        nc.sync.dma_start(dd[:, :, 0:PAD], zpad[:, :, 0:PAD])
                        nc.sync.dma_start(dd[:, :, W + PAD:Wd],
                                          zpad[:, :, 0:PAD + 2])
                    for b in range(NB):
                        g, p0 = b // 8, (b % 8) * 16
                        top = max(0, PAD - b * BR)          # slot rows < image 0
                        if top:
                            nc.sync.dma_start(
                                d_halo[si][g][p0:p0 + 16, 0:top, :],
                                zedge[:, 0:top, :])
                        bot = max(0, (b * BR - PAD + SLOTR) - H)  # rows >= H
                        if bot:
                            nc.sync.dma_start(
                                d_halo[si][g][p0:p0 + 16, SLOTR - bot:SLOTR, :],
                                zedge[:, 0:bot, :])

                # in_conv: large cen tiles, halo'd per-strip x writes
                cps = SR // rpc              # psum chunks per strip pair
                nsp_half = H // (2 * SR)     # strips per half
                for i in range(nchunks):
                    j = i % cps
                    if j == 0:
                        ct = pa.tile([128, cps, 512], BF16, name="ct", tag="cen",
                                     bufs=2)
                        nc.gpsimd.dma_start(
                            ct[0:64], cen_ap[:, i * 512:(i + cps) * 512])
                        nc.gpsimd.dma_start(
                            ct[64:128], cen_ap[:, Ph + i * 512:Ph + (i + cps) * 512])
                        sgx = pa.tile([32, SR, W], BF16, name="sgx", tag="sgx")
                    ps1 = psp.tile([32, 512], F32, name="ps1", tag="ps1")
                    nc.tensor.matmul(ps1, wsb["w_in"], ct[:, j, :], start=True,
                                     stop=True)
                    if i % 2 == 0:
                        nc.scalar.activation(sgx[:, j * rpc:(j + 1) * rpc, :], ps1,
                                             AF.Identity,
                                             bias=wsb["b_in"][:, 0:1], scale=1.0)
                    else:
                        nc.vector.tensor_scalar(
                            sgx[:, j * rpc:(j + 1) * rpc, :], ps1,
                            wsb["b_in"][:, 0:1], None, AL.add)
                    if j == cps - 1:
                        sp = i // cps
                        for half in range(2):
                            s_idx = sp if half == 0 else sp + nsp_half
                            seg = sgx[16 * half:16 * half + 16]
                            nc.sync.dma_start(
                                x_halo[s_idx][:, PAD:PAD + SR, PAD:W + PAD], seg)
                            if s_idx > 0:
                                nc.sync.dma_start(
                                    x_halo[s_idx - 1][:, PAD + SR:SR + 16,
                                                      PAD:W + PAD],
                                    seg[:, 0:PAD, :])
                            if s_idx < H // SR - 1:
                                nc.sync.dma_start(
                                    x_halo[s_idx + 1][:, 0:PAD, PAD:W + PAD],
                                    seg[:, SR - PAD:SR, :])

                # depthwise convs on PE via 8-shift stack
                cpd = SR * W // 512          # psum chunks per strip (row pairs)
                rpk = 512 // W               # rows per chunk
                def load_stack(st):
                    stk = pa.tile([128, SR + 6, W], BF16, name="stk", tag="stk",
                                  bufs=3)
                    for g in range(8):
                        nc.sync.dma_start(
                            stk[16 * g:16 * g + 16],
                            x_halo[st][:, 5:5 + SR + 6, 5 + g:5 + g + W])
                    return stk

                stk_next = load_stack(0)
                for st in range(nstrips):
                    s0 = st * SR
                    stk = stk_next
                    if st + 1 < nstrips:
                        stk_next = load_stack(st + 1)
                    stg = pa.tile([64, SR, W], BF16, name="stg", tag="stg", bufs=3)
                    for k in range(cpd):
                        pd = psp.tile([64, 512], F32, name="pd", tag="pd", bufs=3)
                        for dp in range(7):
                            nc.tensor.matmul(
                                pd, wsb["dwL"][:, 64 * dp:64 * dp + 64],
                                stk[:, dp + k * rpk:dp + k * rpk + rpk, :],
                                start=(dp == 0), stop=(dp == 6))
                        nc.scalar.activation(stg[:, k * rpk:(k + 1) * rpk, :], pd,
                                             AF.Identity, bias=wsb["dwB64"][:, 0:1],
                                             scale=1.0)
                    for si in range(4):
                        # write strip rows into every overlapping band window
                        for b in range(NB):
                            wlo = b * BR - PAD       # image row of slot row 0
                            lo = max(wlo, s0)
                            hi = min(wlo + SLOTR, s0 + SR)
                            if lo >= hi:
                                continue
                            g = b // 8
                            p0 = (b % 8) * 16
                            nc.sync.dma_start(
                                d_halo[si][g][p0:p0 + 16,
                                              lo - wlo:hi - wlo, PAD:W + PAD],
                                stg[16 * si:16 * si + 16, lo - s0:hi - s0, :])

            # ---------------- phase B (lane-split: one branch/tile on
            # GPSIMD "p" lane, three on DVE "v" lane; per-lane tile tags) ----
            LANE_BUFS = {
                "v": {"T": 10, "S": 6, "U": 6, "O": 11, "dt1": 3, "dt2": 2},
                "p": {"T": 8, "S": 4, "U": 4, "O": 10, "dt1": 2, "dt2": 1},
            }
            with tc.tile_pool(name="wk", bufs=1) as wk:
                def scaled(src_ap, scale_ap, ln):
                    t = wk.tile([128, BR, CW], BF16, name="at", tag="AT" + ln,
                                bufs=8 if ln == "v" else 4)
                    nc.scalar.activation(t, src_ap, AF.Copy, bias=0.0,
                                         scale=scale_ap)
                    return t

                def ts2(ln):
                    return nc.vector  # TensorScalarPtr is DVE/ACT-only

                def slab(nm, ln, tg):
                    return wk.tile([128, BR, CW], BF16, name=nm, tag=tg + ln,
                                   bufs=LANE_BUFS[ln][tg])

                for rh in range(RH):
                    for cc in range(CC):
                        c0 = cc * CW
                        pool_si = 0
                        branches = [None] * 4
                        for si in [pool_si] + [x for x in range(4) if x != pool_si]:
                            s = SHIFTS[si]
                            ln = "p" if si == pool_si else "v"
                            eng = nc.vector  # Pool compute not supported by this walrus build
                            dd = d_halo[si][rh]
                            dt1 = wk.tile([128, BR + 16, CW + 16], BF16,
                                          name="dt1", tag="dt1" + ln,
                                          bufs=LANE_BUFS[ln]["dt1"])
                            dt2 = wk.tile([128, BR + 16, CW + 16], BF16,
                                          name="dt2", tag="dt2" + ln,
                                          bufs=LANE_BUFS[ln]["dt2"])
                            nc.sync.dma_start(dt1, dd[:, :, c0:c0 + CW + 16])
                            nc.sync.dma_start(dt2, dd[:, :, c0 + 1:c0 + CW + 17])

                            ctr = dt1[:, 8:8 + BR, 8:8 + CW]
                            T = []
                            for (dy, dx) in OFFSETS:
                                Tk = slab("Tk", ln, "T")
                                ro = 8 + dy * s
                                if dx == 0:
                                    srcv = dt1[:, ro:ro + BR, 8:8 + CW]
                                else:
                                    co = 8 + dx * s - 1  # even (s odd)
                                    srcv = dt2[:, ro:ro + BR, co:co + CW]
                                nc.vector.tensor_tensor(Tk, ctr, srcv, AL.subtract)
                                T.append(Tk)

                            S = []
                            for k in range(4):
                                Sk = slab("Sk", ln, "S")
                                eng.tensor_tensor(Sk, T[k], T[k + 4], AL.add)
                                S.append(Sk)

                            U = []
                            for k in range(4):
                                u = slab("u", ln, "U")
                                ts2(ln).tensor_scalar(
                                    u, S[(k + 1) % 4], wsb["l1w"][:, 4 * si:4 * si + 1],
                                    wsb["l1b"][:, si:si + 1], AL.mult, AL.add)
                                t1 = scaled(S[(k + 3) % 4],
                                            wsb["l1w"][:, 4 * si + 1:4 * si + 2], ln)
                                t2 = scaled(S[(k + 2) % 4],
                                            wsb["l1w"][:, 4 * si + 2:4 * si + 3], ln)
                                nc.gpsimd.dma_start(u, t1, accum_op=AL.add)
                                nc.gpsimd.dma_start(u, t2, accum_op=AL.add)
                                U.append(u)

                            O = []
                            for k in range(8):
                                ok = slab("ok", ln, "O")
                                nc.scalar.activation(
                                    ok, T[(k + 4) % 8], AF.Copy, bias=0.0,
                                    scale=wsb["l1w"][:, 4 * si + 3:4 * si + 4])
                                nc.gpsimd.dma_start(ok, U[k % 4], accum_op=AL.add)
                                eng.tensor_tensor(ok, ok, T[k], AL.mult)
                                O.append(ok)

                            for (a, b) in BATCHER8:
                                mx = slab("mx", ln, "O")
                                eng.tensor_tensor(mx, O[a], O[b], AL.max)
                                eng.tensor_tensor(O[a], O[a], O[b], AL.min)
                                O[b] = mx

                            # rank-weighted dot as a depth-3 tree
                            t0 = slab("t0", ln, "O")
                            ts2(ln).tensor_scalar(
                                t0, O[0], wsb["l2w"][:, 8 * si:8 * si + 1],
                                wsb["l2b"][:, si:si + 1], AL.mult, AL.add)
                            terms = [t0]
                            for r in range(1, 8):
                                terms.append(scaled(
                                    O[r], wsb["l2w"][:, 8 * si + r:8 * si + r + 1],
                                    ln))
                            while len(terms) > 1:
                                nxt = []
                                for i in range(0, len(terms), 2):
                                    acc = slab("acc", ln, "O")
                                    eng.tensor_tensor(acc, terms[i], terms[i + 1],
                                                      AL.add)
                                    nxt.append(acc)
                                terms = nxt
                            branches[si] = terms[0]

                        for (a, b) in SORT4:
                            mx = wk.tile([128, BR, CW], BF16, name="mx4",
                                         tag="BR", bufs=6)
                            nc.vector.tensor_tensor(mx, branches[a], branches[b],
                                                    AL.max)
                            nc.vector.tensor_tensor(branches[a], branches[a],
                                                    branches[b], AL.min)
                            branches[b] = mx

                        y16 = wk.tile([128, BR, CW], BF16, name="y16", tag="Y", bufs=2)
                        nc.vector.tensor_scalar(
                            y16, branches[0], wsb["basew"][:, 0:1], None, AL.mult)
                        for f in range(1, 4):
                            tf = scaled(branches[f], wsb["basew"][:, f:f + 1], "v")
                            nc.vector.tensor_tensor(y16, y16, tf, AL.add)

                        v = wk.tile([128, BR, CW], BF16, name="v", tag="Y", bufs=2)
                        nc.vector.tensor_scalar(v, y16, wsb["bn"][:, 0:1],
                                                wsb["bn"][:, 1:2], AL.mult, AL.add)
                        sg = wk.tile([128, BR, CW], BF16, name="sg", tag="Z", bufs=2)
                        nc.scalar.activation(sg, v, AF.Sigmoid, bias=0.0, scale=1.0)
                        z = wk.tile([128, BR, CW], BF16, name="z", tag="Z", bufs=2)
                        nc.vector.tensor_mul(z, v, sg)

                        FD = BR * CW
                        psf = psp.tile([8, FD], F32, name="psf", tag="psf", bufs=1)
                        if FD <= 512:
                            nc.tensor.matmul(psf, wsb["fin"], z, start=True, stop=True)
                        else:
                            nh = FD // 512
                            rows = BR // nh
                            for h in range(nh):
                                nc.tensor.matmul(
                                    psf[:, h * 512:(h + 1) * 512], wsb["fin"],
                                    z[:, h * rows:(h + 1) * rows, :],
                                    start=True, stop=True)
                        ob = wk.tile([8, BR, CW], F32, name="ob", tag="OB", bufs=1)
                        nc.scalar.activation(ob, psf, AF.Sigmoid,
                                             bias=wsb["finb"][:, 0:1], scale=1.0)
                        ov = out_ap.rearrange("(a b r) (c x) -> a b r c x",
                                              a=RH, b=8, r=BR, c=CC, x=CW)
                        nc.sync.dma_start(ov[rh, :, :, cc, :], ob)
    return nc


def build_program(wdict, H=256, W=256, RH=2, CC=4, SR=32):
    nc = bacc.Bacc("TRN2", target_bir_lowering=False, debug=False)
    cen_d = nc.dram_tensor("cen", [64, H * W], F32, kind="ExternalInput").ap()
    waps = {}
    for nm, shp in WSHAPES.items():
        waps[nm] = nc.dram_tensor(nm, list(shp), F32, kind="ExternalInput").ap()
    out_d = nc.dram_tensor("out", [H, W], F32, kind="ExternalOutput").ap()
    emit(nc, cen_d, waps, out_d, H, W, RH, CC, SR=SR)
    nc.finalize()
    return nc


RESULTS = {}


def kernel(**inputs):
    H = W = 256
    cen = np.ascontiguousarray(np.asarray(inputs["cen"], np.float32))
    B = cen.shape[0]
    packed = pack_weights(inputs)
    nc = build_program(inputs, H=H, W=W, RH=2, CC=4, SR=32)
    in_maps = []
    for i in range(B):
        m = {"cen": np.ascontiguousarray(cen[i].reshape(64, H * W))}
        for nm in WSHAPES:
            m[nm] = packed[nm]
        in_maps.append(m)
    from concourse import bass_utils
    res = bass_utils.run_bass_kernel_spmd(nc, in_maps, core_ids=list(range(B)))
    RESULTS['last'] = res
    out = np.stack([r["out"].reshape(1, H, W) for r in res.results], axis=0)
    return out.astype(np.float32)
